# revision 5
# baseline (speedup 1.0000x reference)
"""Trainium2 Bass kernel for the BiaffineLayer problem (v2).

Math (per batch b):
  out[l, m, c] = x1[l] @ W1[c] + x2[m] @ W2[c]
              + sum_h x1[l,h] * x2[m,h] * W3[c,h]
              + sum_h |x1[l,h] - x2[m,h]| * W4[c,h] + bias[c]
  shapes: x1, x2 [2, 512, 128]; W [25, 512]; bias [25]; out [2, 512, 512, 25]

Sharding: 8 cores = 2 batches x 4 m-blocks of 128 columns. Core (b, mblk)
gets full x1[b] (as [h, l] bf16), its x2 block (as [h, m]), W/bias
replicated; produces out[b, :, m0:m0+128, :] = [512, 128, 25] in bf16.

v2 changes vs the 66.9us baseline (engine-balance redesign):
  - all input casts done on host (x1bf/x2f/negx2/x2bf/wmov shipped ready)
  - D = relu(x1 - x2[m]) tiles split 3 ways: DVE tensor_scalar / ACT
    relu-activation / GPSIMD tensor_scalar (gpsimd was idle before)
  - PSUM allocated as 2 groups of 4 banks; the 4 l-chunks of an m-block
    live in one group -> ONE fused PSUM->SBUF copy (FD=1600) and ONE
    output DMA per m-block (amortizes ScalarE fixed cost + sems 4x)
  - t4 matmuls open each accumulation group (start=True per 25-col
    region) so PE starts before v3/t2row are ready; t3 then bias close it
  - output in bf16 (absmax ~6.2, tolerance 2e-2 -> plenty of margin),
    halves the output DMA bytes
"""

import sys

sys.path.insert(0, "/opt/trn_rl_repo")

from contextlib import ExitStack

import numpy as np
import ml_dtypes

import concourse.bass as bass
import concourse.tile as tile
from concourse import bacc, bass_utils, mybir

F32 = mybir.dt.float32
BF16 = mybir.dt.bfloat16
BF16NP = ml_dtypes.bfloat16

B, L, H, C = 2, 512, 128, 25
MB = 128            # m-block per core
N_CORES = 8
MSUB = 16           # m's per psum chunk
N_MS = MB // MSUB   # 8 chunks over the m-block
LCHUNK = 128
N_LC = L // LCHUNK  # 4 l-chunks
CHUNK_F = MSUB * C  # 400 psum free columns per chunk
BANK_F = 512        # f32 columns per psum bank
GROUP_F = N_LC * BANK_F  # one 4-bank psum group per m-block

# engine assignment for the 16 D-tiles of each m-block
DVE_J = (0, 2, 4, 6, 8, 10, 12, 14)
ACT_J = (1, 5, 9)
GP_J = (3, 7, 11, 13, 15)


def build_kernel(nc: bass.Bass):
    x1bf = nc.dram_tensor("x1bf", (H, L), BF16, kind="ExternalInput").ap()
    x2f = nc.dram_tensor("x2f", (H, MB), F32, kind="ExternalInput").ap()
    nx2f = nc.dram_tensor("nx2f", (H, MB), F32, kind="ExternalInput").ap()
    x2bf = nc.dram_tensor("x2bf", (H, MB), BF16, kind="ExternalInput").ap()
    wmovbf = nc.dram_tensor("wmovbf", (H, 4 * C), BF16, kind="ExternalInput").ap()
    browbf = nc.dram_tensor("browbf", (1, C), BF16, kind="ExternalInput").ap()
    out = nc.dram_tensor("out", (L, MB * C), BF16, kind="ExternalOutput").ap()

    with tile.TileContext(nc) as tc, ExitStack() as ctx:
      const = ctx.enter_context(tc.tile_pool(name="const", bufs=1))
      dpool = ctx.enter_context(tc.tile_pool(name="dpool", bufs=40))
      vpool = ctx.enter_context(tc.tile_pool(name="vpool", bufs=2))
      opool = ctx.enter_context(tc.tile_pool(name="opool", bufs=3))
      psum = ctx.enter_context(tc.tile_pool(name="psum", bufs=2, space="PSUM"))
      dram = ctx.enter_context(tc.tile_pool(name="dram", bufs=1, space="DRAM"))

      # ---- constant loads (all pre-cast on host) ----
      x1bf_s = const.tile([H, L], BF16)
      nc.sync.dma_start(x1bf_s[:], x1bf[:])
      x2f_s = const.tile([H, MB], F32)
      nc.sync.dma_start(x2f_s[:], x2f[:])
      nx2_s = const.tile([H, MB], F32)
      nc.sync.dma_start(nx2_s[:], nx2f[:])
      x2bf_s = const.tile([H, MB], BF16)
      nc.sync.dma_start(x2bf_s[:], x2bf[:])
      wm_s = const.tile([H, 4 * C], BF16)
      nc.sync.dma_start(wm_s[:], wmovbf[:])
      brow_s = const.tile([1, C], BF16)
      nc.sync.dma_start(brow_s[:], browbf[:])
      ones_s = const.tile([1, MB], BF16)
      nc.vector.memset(ones_s[:], 1.0)

      w1m = wm_s[:, 0:C]          # (W1 - W4)^T
      w2p = wm_s[:, C:2 * C]      # (W2 + W4)^T
      w3 = wm_s[:, 2 * C:3 * C]   # W3^T
      w42 = wm_s[:, 3 * C:4 * C]  # 2*W4^T

      # ---- T2B = (t2[m, c] + bias[c]) collapsed to one bf16 row ----
      ps0 = psum.tile([H, GROUP_F], F32, tag="ps")
      nc.tensor.matmul(ps0[:, 0:C], x2bf_s[:], w2p,
                       start=True, stop=False, skip_group_check=True)
      nc.tensor.matmul(ps0[:, 0:C], ones_s[:], brow_s[:],
                       start=False, stop=True, skip_group_check=True)
      t2small = const.tile([MB, C], BF16)
      nc.scalar.copy(t2small[:], ps0[:, 0:C])
      t2d = dram.tile([1, MB * C], BF16)
      nc.sync.dma_start(t2d[:].rearrange("o (m c) -> (o m) c", c=C), t2small[:])
      t2row = const.tile([1, MB * C], BF16)
      nc.sync.dma_start(t2row[:], t2d[:])

      # ---- V3[h, (m, c)] = x2[h,m] * W3T[h,c] + (W1-W4)T[h,c]  (bf16) ----
      VS = 2 * MSUB  # V3 slice width in m's (two m-subs)
      v3 = const.tile([H, MB * C], BF16)
      w3_bc = w3.unsqueeze(1).broadcast_to([H, VS, C])
      w1_bc = w1m.unsqueeze(1).broadcast_to([H, VS, C])

      def v3_prep(vh):
          sl = slice(vh * VS * C, (vh + 1) * VS * C)
          x2_bc = (x2bf_s[:, vh * VS:(vh + 1) * VS]
                   .unsqueeze(2).broadcast_to([H, VS, C]))
          va = vpool.tile([H, VS * C], BF16, tag="v3a")
          va3 = va[:].rearrange("h (m c) -> h m c", c=C)
          nc.vector.tensor_tensor(va3, x2_bc, w3_bc, op=mybir.AluOpType.mult)
          nc.vector.tensor_tensor(v3[:, sl].rearrange("h (m c) -> h m c", c=C),
                                  va3, w1_bc, op=mybir.AluOpType.add)

      v3_prep(0)

      # ---- main loop over m-blocks ----
      for ms in range(N_MS):
          if ms % 2 == 0 and ms // 2 + 1 < MB // VS:
              v3_prep(ms // 2 + 1)
          dts = []
          for j in range(MSUB):
              m = ms * MSUB + j
              dt_ = dpool.tile([H, L], BF16, tag="d")
              if j in ACT_J:
                  nc.scalar.activation(
                      dt_[:], x1bf_s[:], mybir.ActivationFunctionType.Relu,
                      bias=nx2_s[:, m:m + 1], scale=1.0)
              elif j in GP_J:
                  nc.gpsimd.tensor_scalar(
                      dt_[:], x1bf_s[:], x2f_s[:, m:m + 1], 0.0,
                      op0=mybir.AluOpType.subtract, op1=mybir.AluOpType.max)
              else:
                  nc.vector.tensor_scalar(
                      dt_[:], x1bf_s[:], x2f_s[:, m:m + 1], 0.0,
                      op0=mybir.AluOpType.subtract, op1=mybir.AluOpType.max)
              dts.append(dt_)
          ps = psum.tile([H, GROUP_F], F32, tag="ps")
          for lc in range(N_LC):
              base = lc * BANK_F
              for j in range(MSUB):
                  # start=True zeroes the whole 2KB psum bank (ZERO_REGION),
                  # so only the bank's first matmul may set it.
                  nc.tensor.matmul(
                      ps[:, base + j * C: base + (j + 1) * C],
                      dts[j][:, lc * LCHUNK:(lc + 1) * LCHUNK], w42,
                      start=(j == 0), stop=False, skip_group_check=True)
              nc.tensor.matmul(
                  ps[:, base: base + CHUNK_F],
                  x1bf_s[:, lc * LCHUNK:(lc + 1) * LCHUNK],
                  v3[:, ms * CHUNK_F:(ms + 1) * CHUNK_F],
                  start=False, stop=False, skip_group_check=True)
              nc.tensor.matmul(
                  ps[:, base: base + CHUNK_F], ones_s[:],
                  t2row[:, ms * CHUNK_F:(ms + 1) * CHUNK_F],
                  start=False, stop=True, skip_group_check=True)
          o_sb = opool.tile([LCHUNK, N_LC * CHUNK_F], BF16)
          nc.scalar.copy(
              o_sb[:].rearrange("p (g x) -> p g x", x=CHUNK_F),
              ps[:].rearrange("p (g x) -> p g x", x=BANK_F)[:, :, 0:CHUNK_F])
          nc.sync.dma_start(
              out[:, ms * CHUNK_F:(ms + 1) * CHUNK_F]
              .rearrange("(g p) x -> p g x", p=LCHUNK),
              o_sb[:].rearrange("p (g x) -> p g x", x=CHUNK_F))
    return nc


_COMPILED = {}


def _get_compiled():
    if "nc" not in _COMPILED:
        nc = bacc.Bacc("TRN2", target_bir_lowering=False, debug=False,
                       num_devices=N_CORES)
        build_kernel(nc)
        nc.compile()
        _COMPILED["nc"] = nc
    return _COMPILED["nc"]


def make_in_maps(x1, x2, W, b):
    W1, W2, W3, W4 = (W[:, 0:H], W[:, H:2 * H], W[:, 2 * H:3 * H],
                      W[:, 3 * H:4 * H])
    wmov = np.ascontiguousarray(
        np.concatenate([(W1 - W4).T, (W2 + W4).T, W3.T, (2.0 * W4).T], axis=1)
    ).astype(BF16NP)
    brow = np.ascontiguousarray(b.reshape(1, C)).astype(BF16NP)
    in_maps = []
    for cid in range(N_CORES):
        bb, mblk = cid // 4, cid % 4
        m0 = mblk * MB
        x1t = np.ascontiguousarray(x1[bb].T, dtype=np.float32)
        x2t = np.ascontiguousarray(x2[bb, m0:m0 + MB].T, dtype=np.float32)
        in_maps.append({
            "x1bf": x1t.astype(BF16NP),
            "x2f": x2t,
            "nx2f": np.ascontiguousarray(-x2t),
            "x2bf": x2t.astype(BF16NP),
            "wmovbf": wmov,
            "browbf": brow,
        })
    return in_maps


def run_on_device(x1, x2, W, b, trace=False, trace_kwargs=None):
    nc = _get_compiled()
    in_maps = make_in_maps(x1, x2, W, b)
    res = bass_utils.run_bass_kernel_spmd(
        nc, in_maps, core_ids=list(range(N_CORES)), trace=trace,
        **(trace_kwargs or {}))
    full = np.empty((B, L, L, C), dtype=np.float32)
    for cid in range(N_CORES):
        bb, mblk = cid // 4, cid % 4
        m0 = mblk * MB
        full[bb, :, m0:m0 + MB, :] = (
            np.asarray(res.results[cid]["out"])
            .astype(np.float32).reshape(L, MB, C))
    return full, res


def kernel(x1, x2, W, b):
    x1 = np.asarray(x1, dtype=np.float32)
    x2 = np.asarray(x2, dtype=np.float32)
    W = np.asarray(W, dtype=np.float32)
    b = np.asarray(b, dtype=np.float32)
    full, _ = run_on_device(x1, x2, W, b, trace=False)
    return full


# revision 10
# speedup vs baseline: 4.0743x; 4.0743x over previous
"""Trainium2 Bass kernel for the BiaffineLayer problem (v2).

Math (per batch b):
  out[l, m, c] = x1[l] @ W1[c] + x2[m] @ W2[c]
              + sum_h x1[l,h] * x2[m,h] * W3[c,h]
              + sum_h |x1[l,h] - x2[m,h]| * W4[c,h] + bias[c]
  shapes: x1, x2 [2, 512, 128]; W [25, 512]; bias [25]; out [2, 512, 512, 25]

Sharding: 8 cores = 2 batches x 4 m-blocks of 128 columns. Core (b, mblk)
gets full x1[b] (as [h, l] bf16), its x2 block (as [h, m]), W/bias
replicated; produces out[b, :, m0:m0+128, :] = [512, 128, 25] in bf16.

v2 changes vs the 66.9us baseline (engine-balance redesign):
  - all input casts done on host (x1bf/x2f/negx2/x2bf/wmov shipped ready)
  - D = relu(x1 - x2[m]) tiles split 3 ways: DVE tensor_scalar / ACT
    relu-activation / GPSIMD tensor_scalar (gpsimd was idle before)
  - PSUM allocated as 2 groups of 4 banks; the 4 l-chunks of an m-block
    live in one group -> ONE fused PSUM->SBUF copy (FD=1600) and ONE
    output DMA per m-block (amortizes ScalarE fixed cost + sems 4x)
  - t4 matmuls open each accumulation group (start=True per 25-col
    region) so PE starts before v3/t2row are ready; t3 then bias close it
  - output in bf16 (absmax ~6.2, tolerance 2e-2 -> plenty of margin),
    halves the output DMA bytes
"""

import sys

sys.path.insert(0, "/opt/trn_rl_repo")

from contextlib import ExitStack

import numpy as np
import ml_dtypes

import concourse.bass as bass
import concourse.tile as tile
from concourse import bacc, bass_utils, mybir

F32 = mybir.dt.float32
BF16 = mybir.dt.bfloat16
BF16NP = ml_dtypes.bfloat16

B, L, H, C = 2, 512, 128, 25
MB = 128            # m-block per core
N_CORES = 8
MSUB = 16           # m's per psum chunk
N_MS = MB // MSUB   # 8 chunks over the m-block
LCHUNK = 128
N_LC = L // LCHUNK  # 4 l-chunks
CHUNK_F = MSUB * C  # 400 psum free columns per chunk
BANK_F = 512        # f32 columns per psum bank
GROUP_F = N_LC * BANK_F  # one 4-bank psum group per m-block

# engine assignment for the 16 D-tiles of each m-block (gpsimd compute is
# ~22x slower than DVE and also stalls concurrent DVE ops -> not used)
ACT_J = (5, 11)          # ACT D-tiles every block
ACT_J_EXTRA = 14         # + one more on even blocks


def build_kernel(nc: bass.Bass):
    x1bf = nc.dram_tensor("x1bf", (H, L), BF16, kind="ExternalInput").ap()
    x2f = nc.dram_tensor("x2f", (H, MB), F32, kind="ExternalInput").ap()
    nx2f = nc.dram_tensor("nx2f", (H, MB), F32, kind="ExternalInput").ap()
    x2bf = nc.dram_tensor("x2bf", (H, MB), BF16, kind="ExternalInput").ap()
    wmovbf = nc.dram_tensor("wmovbf", (H, 4 * C), BF16, kind="ExternalInput").ap()
    w3f = nc.dram_tensor("w3f", (H, C), F32, kind="ExternalInput").ap()
    w1mf = nc.dram_tensor("w1mf", (H, C), F32, kind="ExternalInput").ap()
    browbf = nc.dram_tensor("browbf", (1, C), BF16, kind="ExternalInput").ap()
    out = nc.dram_tensor("out", (L, MB * C), BF16, kind="ExternalOutput").ap()

    with tile.TileContext(nc) as tc, ExitStack() as ctx:
      const = ctx.enter_context(tc.tile_pool(name="const", bufs=1))
      dpool = ctx.enter_context(tc.tile_pool(name="dpool", bufs=40))
      vpool = ctx.enter_context(tc.tile_pool(name="vpool", bufs=2))
      opool = ctx.enter_context(tc.tile_pool(name="opool", bufs=3))
      psum = ctx.enter_context(tc.tile_pool(name="psum", bufs=2, space="PSUM"))
      dram = ctx.enter_context(tc.tile_pool(name="dram", bufs=1, space="DRAM"))

      # ---- constant loads (all pre-cast on host); t2-path tensors first ----
      x2bf_s = const.tile([H, MB], BF16)
      nc.sync.dma_start(x2bf_s[:], x2bf[:])
      wm_s = const.tile([H, 4 * C], BF16)
      nc.sync.dma_start(wm_s[:], wmovbf[:])
      brow_s = const.tile([1, C], BF16)
      nc.sync.dma_start(brow_s[:], browbf[:])
      x1bf_s = const.tile([H, L], BF16)
      nc.sync.dma_start(x1bf_s[:], x1bf[:])
      x2f_s = const.tile([H, MB], F32)
      nc.sync.dma_start(x2f_s[:], x2f[:])
      nx2_s = const.tile([H, MB], F32)
      nc.sync.dma_start(nx2_s[:], nx2f[:])
      w3f_s = const.tile([H, C], F32)
      nc.sync.dma_start(w3f_s[:], w3f[:])
      w1mf_s = const.tile([H, C], F32)
      nc.sync.dma_start(w1mf_s[:], w1mf[:])
      ones_s = const.tile([1, MB], BF16)
      nc.vector.memset(ones_s[:], 1.0)

      w1m = wm_s[:, 0:C]          # (W1 - W4)^T
      w2p = wm_s[:, C:2 * C]      # (W2 + W4)^T
      w3 = wm_s[:, 2 * C:3 * C]   # W3^T
      w42 = wm_s[:, 3 * C:4 * C]  # 2*W4^T

      # ---- T2B = (t2[m, c] + bias[c]) collapsed to one bf16 row ----
      ps0 = psum.tile([H, GROUP_F], F32, tag="ps")
      nc.tensor.matmul(ps0[:, 0:C], x2bf_s[:], w2p,
                       start=True, stop=False, skip_group_check=True)
      nc.tensor.matmul(ps0[:, 0:C], ones_s[:], brow_s[:],
                       start=False, stop=True, skip_group_check=True)
      t2small = const.tile([MB, C], BF16)
      nc.scalar.copy(t2small[:], ps0[:, 0:C])
      t2d = dram.tile([1, MB * C], BF16)
      nc.sync.dma_start(t2d[:].rearrange("o (m c) -> (o m) c", c=C), t2small[:])
      t2row = const.tile([1, MB * C], BF16)
      nc.sync.dma_start(t2row[:], t2d[:])

      # ---- V3[h, (m, c)] = x2[h,m] * W3T[h,c] + (W1-W4)T[h,c]  (bf16) ----
      # Hybrid prep: m in [0, VS) via two DVE tensor_tensors (ready fast,
      # unblocks the first two m-blocks' t3); m in [VS, MB) via 25 per-c
      # ACT Identity activations (scale=w3 col, bias=w1m col).
      VS = 2 * MSUB
      v3 = const.tile([H, MB * C], BF16)
      v3_3d = v3[:].rearrange("h (m c) -> h m c", c=C)
      w3_bc = w3.unsqueeze(1).broadcast_to([H, VS, C])
      w1_bc = w1m.unsqueeze(1).broadcast_to([H, VS, C])
      x2_bc = x2bf_s[:, 0:VS].unsqueeze(2).broadcast_to([H, VS, C])
      va = vpool.tile([H, VS * C], BF16, tag="v3a")
      va3 = va[:].rearrange("h (m c) -> h m c", c=C)
      nc.vector.tensor_tensor(va3, x2_bc, w3_bc, op=mybir.AluOpType.mult)
      nc.vector.tensor_tensor(v3_3d[:, 0:VS, :], va3, w1_bc,
                              op=mybir.AluOpType.add)
      for cc in range(C):
          nc.scalar.activation(
              v3_3d[:, VS:MB, cc], x2bf_s[:, VS:MB],
              mybir.ActivationFunctionType.Identity,
              bias=w1mf_s[:, cc:cc + 1], scale=w3f_s[:, cc:cc + 1])

      # ---- main loop over m-blocks ----
      for ms in range(N_MS):
          dts = []
          for j in range(MSUB):
              m = ms * MSUB + j
              dt_ = dpool.tile([H, L], BF16, tag="d")
              if j in ACT_J or (j == ACT_J_EXTRA and ms % 2 == 0):
                  nc.scalar.activation(
                      dt_[:], x1bf_s[:], mybir.ActivationFunctionType.Relu,
                      bias=nx2_s[:, m:m + 1], scale=1.0)
              else:
                  nc.vector.tensor_scalar(
                      dt_[:], x1bf_s[:], x2f_s[:, m:m + 1], 0.0,
                      op0=mybir.AluOpType.subtract, op1=mybir.AluOpType.max)
              dts.append(dt_)
          ps = psum.tile([H, GROUP_F], F32, tag="ps")
          for lc in range(N_LC):
              base = lc * BANK_F
              for j in range(MSUB):
                  # start=True zeroes the whole 2KB psum bank (ZERO_REGION),
                  # so only the bank's first matmul may set it.
                  nc.tensor.matmul(
                      ps[:, base + j * C: base + (j + 1) * C],
                      dts[j][:, lc * LCHUNK:(lc + 1) * LCHUNK], w42,
                      start=(j == 0), stop=False, skip_group_check=True)
              nc.tensor.matmul(
                  ps[:, base: base + CHUNK_F],
                  x1bf_s[:, lc * LCHUNK:(lc + 1) * LCHUNK],
                  v3[:, ms * CHUNK_F:(ms + 1) * CHUNK_F],
                  start=False, stop=False, skip_group_check=True)
              nc.tensor.matmul(
                  ps[:, base: base + CHUNK_F], ones_s[:],
                  t2row[:, ms * CHUNK_F:(ms + 1) * CHUNK_F],
                  start=False, stop=True, skip_group_check=True)
          o_sb = opool.tile([LCHUNK, N_LC * CHUNK_F], BF16)
          nc.scalar.copy(
              o_sb[:].rearrange("p (g x) -> p g x", x=CHUNK_F),
              ps[:].rearrange("p (g x) -> p g x", x=BANK_F)[:, :, 0:CHUNK_F])
          nc.sync.dma_start(
              out[:, ms * CHUNK_F:(ms + 1) * CHUNK_F]
              .rearrange("(g p) x -> p g x", p=LCHUNK),
              o_sb[:].rearrange("p (g x) -> p g x", x=CHUNK_F))
    return nc


_COMPILED = {}


def _get_compiled():
    if "nc" not in _COMPILED:
        nc = bacc.Bacc("TRN2", target_bir_lowering=False, debug=False,
                       num_devices=N_CORES)
        build_kernel(nc)
        nc.compile()
        _COMPILED["nc"] = nc
    return _COMPILED["nc"]


def make_in_maps(x1, x2, W, b):
    W1, W2, W3, W4 = (W[:, 0:H], W[:, H:2 * H], W[:, 2 * H:3 * H],
                      W[:, 3 * H:4 * H])
    wmov = np.ascontiguousarray(
        np.concatenate([(W1 - W4).T, (W2 + W4).T, W3.T, (2.0 * W4).T], axis=1)
    ).astype(BF16NP)
    brow = np.ascontiguousarray(b.reshape(1, C)).astype(BF16NP)
    in_maps = []
    for cid in range(N_CORES):
        bb, mblk = cid // 4, cid % 4
        m0 = mblk * MB
        x1t = np.ascontiguousarray(x1[bb].T, dtype=np.float32)
        x2t = np.ascontiguousarray(x2[bb, m0:m0 + MB].T, dtype=np.float32)
        in_maps.append({
            "x1bf": x1t.astype(BF16NP),
            "x2f": x2t,
            "nx2f": np.ascontiguousarray(-x2t),
            "x2bf": x2t.astype(BF16NP),
            "wmovbf": wmov,
            "w3f": np.ascontiguousarray(W3.T, dtype=np.float32),
            "w1mf": np.ascontiguousarray((W1 - W4).T, dtype=np.float32),
            "browbf": brow,
        })
    return in_maps


def run_on_device(x1, x2, W, b, trace=False, trace_kwargs=None):
    nc = _get_compiled()
    in_maps = make_in_maps(x1, x2, W, b)
    res = bass_utils.run_bass_kernel_spmd(
        nc, in_maps, core_ids=list(range(N_CORES)), trace=trace,
        **(trace_kwargs or {}))
    full = np.empty((B, L, L, C), dtype=np.float32)
    for cid in range(N_CORES):
        bb, mblk = cid // 4, cid % 4
        m0 = mblk * MB
        full[bb, :, m0:m0 + MB, :] = (
            np.asarray(res.results[cid]["out"])
            .astype(np.float32).reshape(L, MB, C))
    return full, res


def kernel(x1, x2, W, b):
    x1 = np.asarray(x1, dtype=np.float32)
    x2 = np.asarray(x2, dtype=np.float32)
    W = np.asarray(W, dtype=np.float32)
    b = np.asarray(b, dtype=np.float32)
    full, _ = run_on_device(x1, x2, W, b, trace=False)
    return full


# revision 12
# speedup vs baseline: 4.3412x; 1.0655x over previous
"""Trainium2 Bass kernel for the BiaffineLayer problem (v2).

Math (per batch b):
  out[l, m, c] = x1[l] @ W1[c] + x2[m] @ W2[c]
              + sum_h x1[l,h] * x2[m,h] * W3[c,h]
              + sum_h |x1[l,h] - x2[m,h]| * W4[c,h] + bias[c]
  shapes: x1, x2 [2, 512, 128]; W [25, 512]; bias [25]; out [2, 512, 512, 25]

Sharding: 8 cores = 2 batches x 4 m-blocks of 128 columns. Core (b, mblk)
gets full x1[b] (as [h, l] bf16), its x2 block (as [h, m]), W/bias
replicated; produces out[b, :, m0:m0+128, :] = [512, 128, 25] in bf16.

v2 changes vs the 66.9us baseline (engine-balance redesign):
  - all input casts done on host (x1bf/x2f/negx2/x2bf/wmov shipped ready)
  - D = relu(x1 - x2[m]) tiles split 3 ways: DVE tensor_scalar / ACT
    relu-activation / GPSIMD tensor_scalar (gpsimd was idle before)
  - PSUM allocated as 2 groups of 4 banks; the 4 l-chunks of an m-block
    live in one group -> ONE fused PSUM->SBUF copy (FD=1600) and ONE
    output DMA per m-block (amortizes ScalarE fixed cost + sems 4x)
  - t4 matmuls open each accumulation group (start=True per 25-col
    region) so PE starts before v3/t2row are ready; t3 then bias close it
  - output in bf16 (absmax ~6.2, tolerance 2e-2 -> plenty of margin),
    halves the output DMA bytes
"""

import sys

sys.path.insert(0, "/opt/trn_rl_repo")

from contextlib import ExitStack

import numpy as np
import ml_dtypes

import concourse.bass as bass
import concourse.tile as tile
from concourse import bacc, bass_utils, mybir

F32 = mybir.dt.float32
BF16 = mybir.dt.bfloat16
BF16NP = ml_dtypes.bfloat16

B, L, H, C = 2, 512, 128, 25
MB = 128            # m-block per core
N_CORES = 8
MSUB = 16           # m's per psum chunk
N_MS = MB // MSUB   # 8 chunks over the m-block
LCHUNK = 128
N_LC = L // LCHUNK  # 4 l-chunks
CHUNK_F = MSUB * C  # 400 psum free columns per chunk
BANK_F = 512        # f32 columns per psum bank
GROUP_F = N_LC * BANK_F  # one 4-bank psum group per m-block

# engine assignment for the 16 D-tiles of each m-block (gpsimd compute is
# ~22x slower than DVE and also stalls concurrent DVE ops -> not used).
# Measured rates: DVE tensor_scalar 348ns, ACT relu-activation 710ns.
ACT_J = (3, 7, 11, 14)   # ACT D-tiles every block
ACT_J_EXTRA = 15         # + one more on even blocks


def build_kernel(nc: bass.Bass):
    x1bf = nc.dram_tensor("x1bf", (H, L), BF16, kind="ExternalInput").ap()
    x2f = nc.dram_tensor("x2f", (H, MB), F32, kind="ExternalInput").ap()
    nx2f = nc.dram_tensor("nx2f", (H, MB), F32, kind="ExternalInput").ap()
    x2bf = nc.dram_tensor("x2bf", (H, MB), BF16, kind="ExternalInput").ap()
    wmovbf = nc.dram_tensor("wmovbf", (H, 4 * C), BF16, kind="ExternalInput").ap()
    w3f = nc.dram_tensor("w3f", (H, C), F32, kind="ExternalInput").ap()
    w1mf = nc.dram_tensor("w1mf", (H, C), F32, kind="ExternalInput").ap()
    browbf = nc.dram_tensor("browbf", (1, C), BF16, kind="ExternalInput").ap()
    out = nc.dram_tensor("out", (L, MB * C), BF16, kind="ExternalOutput").ap()

    with tile.TileContext(nc) as tc, ExitStack() as ctx:
      const = ctx.enter_context(tc.tile_pool(name="const", bufs=1))
      dpool = ctx.enter_context(tc.tile_pool(name="dpool", bufs=40))
      vpool = ctx.enter_context(tc.tile_pool(name="vpool", bufs=2))
      opool = ctx.enter_context(tc.tile_pool(name="opool", bufs=3))
      psum = ctx.enter_context(tc.tile_pool(name="psum", bufs=2, space="PSUM"))
      dram = ctx.enter_context(tc.tile_pool(name="dram", bufs=1, space="DRAM"))

      # ---- constant loads (all pre-cast on host); t2-path tensors first ----
      x2bf_s = const.tile([H, MB], BF16)
      nc.sync.dma_start(x2bf_s[:], x2bf[:])
      wm_s = const.tile([H, 4 * C], BF16)
      nc.sync.dma_start(wm_s[:], wmovbf[:])
      brow_s = const.tile([1, C], BF16)
      nc.sync.dma_start(brow_s[:], browbf[:])
      x1bf_s = const.tile([H, L], BF16)
      nc.sync.dma_start(x1bf_s[:], x1bf[:])
      x2f_s = const.tile([H, MB], F32)
      nc.sync.dma_start(x2f_s[:], x2f[:])
      nx2_s = const.tile([H, MB], F32)
      nc.sync.dma_start(nx2_s[:], nx2f[:])
      w3f_s = const.tile([H, C], F32)
      nc.sync.dma_start(w3f_s[:], w3f[:])
      w1mf_s = const.tile([H, C], F32)
      nc.sync.dma_start(w1mf_s[:], w1mf[:])
      ones_s = const.tile([1, MB], BF16)
      nc.vector.memset(ones_s[:], 1.0)

      w1m = wm_s[:, 0:C]          # (W1 - W4)^T
      w2p = wm_s[:, C:2 * C]      # (W2 + W4)^T
      w3 = wm_s[:, 2 * C:3 * C]   # W3^T
      w42 = wm_s[:, 3 * C:4 * C]  # 2*W4^T

      # ---- T2B = (t2[m, c] + bias[c]) collapsed to one bf16 row ----
      ps0 = psum.tile([H, GROUP_F], F32, tag="ps")
      nc.tensor.matmul(ps0[:, 0:C], x2bf_s[:], w2p,
                       start=True, stop=False, skip_group_check=True)
      nc.tensor.matmul(ps0[:, 0:C], ones_s[:], brow_s[:],
                       start=False, stop=True, skip_group_check=True)
      t2small = const.tile([MB, C], BF16)
      nc.scalar.copy(t2small[:], ps0[:, 0:C])
      t2d = dram.tile([1, MB * C], BF16)
      nc.sync.dma_start(t2d[:].rearrange("o (m c) -> (o m) c", c=C), t2small[:])
      t2row = const.tile([1, MB * C], BF16)
      nc.sync.dma_start(t2row[:], t2d[:])

      # ---- V3[h, (m, c)] = x2[h,m] * W3T[h,c] + (W1-W4)T[h,c]  (bf16) ----
      # Two DVE tensor_tensors per VS-wide m slice, emitted two blocks
      # ahead of use (strided-out ACT Identity ops measured ~10x slower
      # than modeled -> v3 stays on DVE).
      VS = 2 * MSUB
      v3 = const.tile([H, MB * C], BF16)
      w3_bc = w3.unsqueeze(1).broadcast_to([H, VS, C])
      w1_bc = w1m.unsqueeze(1).broadcast_to([H, VS, C])

      def v3_prep(vh):
          sl = slice(vh * VS * C, (vh + 1) * VS * C)
          x2_bc = (x2bf_s[:, vh * VS:(vh + 1) * VS]
                   .unsqueeze(2).broadcast_to([H, VS, C]))
          va = vpool.tile([H, VS * C], BF16, tag="v3a")
          va3 = va[:].rearrange("h (m c) -> h m c", c=C)
          nc.vector.tensor_tensor(va3, x2_bc, w3_bc, op=mybir.AluOpType.mult)
          nc.vector.tensor_tensor(v3[:, sl].rearrange("h (m c) -> h m c", c=C),
                                  va3, w1_bc, op=mybir.AluOpType.add)

      v3_prep(0)

      # ---- main loop over m-blocks ----
      for ms in range(N_MS):
          if ms % 2 == 0 and ms // 2 + 1 < MB // VS:
              v3_prep(ms // 2 + 1)
          dts = []
          for j in range(MSUB):
              m = ms * MSUB + j
              dt_ = dpool.tile([H, L], BF16, tag="d")
              if j in ACT_J or (j == ACT_J_EXTRA and ms % 2 == 0):
                  nc.scalar.activation(
                      dt_[:], x1bf_s[:], mybir.ActivationFunctionType.Relu,
                      bias=nx2_s[:, m:m + 1], scale=1.0)
              else:
                  nc.vector.tensor_scalar(
                      dt_[:], x1bf_s[:], x2f_s[:, m:m + 1], 0.0,
                      op0=mybir.AluOpType.subtract, op1=mybir.AluOpType.max)
              dts.append(dt_)
          ps = psum.tile([H, GROUP_F], F32, tag="ps")
          for lc in range(N_LC):
              base = lc * BANK_F
              for j in range(MSUB):
                  # start=True zeroes the whole 2KB psum bank (ZERO_REGION),
                  # so only the bank's first matmul may set it.
                  nc.tensor.matmul(
                      ps[:, base + j * C: base + (j + 1) * C],
                      dts[j][:, lc * LCHUNK:(lc + 1) * LCHUNK], w42,
                      start=(j == 0), stop=False, skip_group_check=True)
              nc.tensor.matmul(
                  ps[:, base: base + CHUNK_F],
                  x1bf_s[:, lc * LCHUNK:(lc + 1) * LCHUNK],
                  v3[:, ms * CHUNK_F:(ms + 1) * CHUNK_F],
                  start=False, stop=False, skip_group_check=True)
              nc.tensor.matmul(
                  ps[:, base: base + CHUNK_F], ones_s[:],
                  t2row[:, ms * CHUNK_F:(ms + 1) * CHUNK_F],
                  start=False, stop=True, skip_group_check=True)
          o_sb = opool.tile([LCHUNK, N_LC * CHUNK_F], BF16)
          nc.scalar.copy(
              o_sb[:].rearrange("p (g x) -> p g x", x=CHUNK_F),
              ps[:].rearrange("p (g x) -> p g x", x=BANK_F)[:, :, 0:CHUNK_F])
          nc.sync.dma_start(
              out[:, ms * CHUNK_F:(ms + 1) * CHUNK_F]
              .rearrange("(g p) x -> p g x", p=LCHUNK),
              o_sb[:].rearrange("p (g x) -> p g x", x=CHUNK_F))
    return nc


_COMPILED = {}


def _get_compiled():
    if "nc" not in _COMPILED:
        nc = bacc.Bacc("TRN2", target_bir_lowering=False, debug=False,
                       num_devices=N_CORES)
        build_kernel(nc)
        nc.compile()
        _COMPILED["nc"] = nc
    return _COMPILED["nc"]


def make_in_maps(x1, x2, W, b):
    W1, W2, W3, W4 = (W[:, 0:H], W[:, H:2 * H], W[:, 2 * H:3 * H],
                      W[:, 3 * H:4 * H])
    wmov = np.ascontiguousarray(
        np.concatenate([(W1 - W4).T, (W2 + W4).T, W3.T, (2.0 * W4).T], axis=1)
    ).astype(BF16NP)
    brow = np.ascontiguousarray(b.reshape(1, C)).astype(BF16NP)
    in_maps = []
    for cid in range(N_CORES):
        bb, mblk = cid // 4, cid % 4
        m0 = mblk * MB
        x1t = np.ascontiguousarray(x1[bb].T, dtype=np.float32)
        x2t = np.ascontiguousarray(x2[bb, m0:m0 + MB].T, dtype=np.float32)
        in_maps.append({
            "x1bf": x1t.astype(BF16NP),
            "x2f": x2t,
            "nx2f": np.ascontiguousarray(-x2t),
            "x2bf": x2t.astype(BF16NP),
            "wmovbf": wmov,
            "w3f": np.ascontiguousarray(W3.T, dtype=np.float32),
            "w1mf": np.ascontiguousarray((W1 - W4).T, dtype=np.float32),
            "browbf": brow,
        })
    return in_maps


def run_on_device(x1, x2, W, b, trace=False, trace_kwargs=None):
    nc = _get_compiled()
    in_maps = make_in_maps(x1, x2, W, b)
    res = bass_utils.run_bass_kernel_spmd(
        nc, in_maps, core_ids=list(range(N_CORES)), trace=trace,
        **(trace_kwargs or {}))
    full = np.empty((B, L, L, C), dtype=np.float32)
    for cid in range(N_CORES):
        bb, mblk = cid // 4, cid % 4
        m0 = mblk * MB
        full[bb, :, m0:m0 + MB, :] = (
            np.asarray(res.results[cid]["out"])
            .astype(np.float32).reshape(L, MB, C))
    return full, res


def kernel(x1, x2, W, b):
    x1 = np.asarray(x1, dtype=np.float32)
    x2 = np.asarray(x2, dtype=np.float32)
    W = np.asarray(W, dtype=np.float32)
    b = np.asarray(b, dtype=np.float32)
    full, _ = run_on_device(x1, x2, W, b, trace=False)
    return full
